# revision 33
# baseline (speedup 1.0000x reference)
"""Trainium2 Bass kernel: 16-head causal MHA (B=2, S=2048, hidden=1024).

Sharding (data + head parallel over 8 cores): core c handles batch c//4
and heads [4*(c%4), 4*(c%4)+4). Each core computes its q/k/v projections,
causal attention for its 4 heads, and a partial o-projection restricted to
its head columns. The host sums the 4 partials per batch (the post-o_proj
all-reduce, done host-side during gather) and adds the exactly-linear bias
terms (bv @ wo.T + bo). bq/bk are applied on device via rank-1 bias
matmuls.

Layout strategy avoids all on-device transposes:
  - host passes hidden pre-transposed xT (HID, S) so projections produce
    qT/kT [d, s] directly and v in [t, d];
  - scores are computed transposed, scoresT[t, s] = kT-slice.T @ qT-slice,
    so the softmax-normalization sums over t arrive for free by augmenting
    v with a ones column in the PV matmul (row 64 of the PV output is the
    softmax denominator);
  - the per-column reciprocal is broadcast across partitions with a K=1
    matmul against a ones vector.

All matmuls run as float32r (full PE rate at free-dim >= 256, ~2x fp32
matmul throughput, ~16-bit-mantissa precision). The BIR verifier requires
every fp32r matmul operand to be produced by a compute op that rounds to
fp32r, so DMA loads land in a staging tile and are converted by DVE/ACT
copies. Softmax skips the max-subtraction: at this problem's scale the
scores are O(1) so exp is safe in fp32, and exp(s)*mask/sum equals
softmax(where(mask, s, -inf)) exactly.
"""

import numpy as np

import concourse.bass as bass
import concourse.mybir as mybir
import concourse.tile as tile
from concourse import bacc
from concourse.bass_utils import run_bass_kernel_spmd

B, S, HID = 2, 2048, 1024
NH, HD = 16, 64
N_CORES = 8
HPC = 4            # heads per core
DPC = HPC * HD     # 256 head-dims per core
SC = 512           # s-chunk (matmul free dim)
NSC = S // SC      # 4
TT = 128           # t-tile (partitions)
NTT = S // TT      # 16
NKT = HID // 128   # 8 contraction tiles for the projections

F32 = mybir.dt.float32
F32R = mybir.dt.float32r
EXP = mybir.ActivationFunctionType.Exp


def _build(causal: bool, has_bias: bool = True):
    nc = bacc.Bacc(
        "TRN2",
        target_bir_lowering=False,
        debug=False,
        enable_asserts=False,
        num_devices=N_CORES,
    )
    xT = nc.dram_tensor("xT", [HID, S], F32, kind="ExternalInput").ap()
    wqT = nc.dram_tensor("wqT", [HID, DPC], F32, kind="ExternalInput").ap()
    wkT = nc.dram_tensor("wkT", [HID, DPC], F32, kind="ExternalInput").ap()
    wvT = nc.dram_tensor("wvT", [HID, DPC], F32, kind="ExternalInput").ap()
    woT = nc.dram_tensor("woT", [DPC, HID], F32, kind="ExternalInput").ap()
    bqr = nc.dram_tensor("bq_r", [1, DPC], F32, kind="ExternalInput").ap()
    bkr = nc.dram_tensor("bk_r", [1, DPC], F32, kind="ExternalInput").ap()
    mskd = nc.dram_tensor("mask_tri", [TT, TT], F32, kind="ExternalInput").ap()
    outT = nc.dram_tensor("outT", [HID, S], F32, kind="ExternalOutput").ap()

    S2 = S // 2          # 1024: columns per half
    NS2 = NSC // 2       # 2 s-chunks per half
    NT2 = NTT // 2       # 8 t-tiles per half
    WAVE = 8             # t-tiles per exp wave

    ctx_lp = nc.allow_low_precision(reason="fp32r matmul pipeline (deliberate)")
    ctx_lp.__enter__()
    with tile.TileContext(nc) as tc:
        with (
            tc.tile_pool(name="persist", bufs=1) as pp,
            tc.tile_pool(name="xpool", bufs=1) as xp,
            tc.tile_pool(name="wpool", bufs=1) as wp,
            tc.tile_pool(name="stage", bufs=3) as sp,
            tc.tile_pool(name="expbuf", bufs=2) as e_pool,
            tc.tile_pool(name="attn", bufs=2) as attn_pool,
            tc.tile_pool(name="osb", bufs=3) as o_pool,
            tc.tile_pool(name="small", bufs=2) as sm_pool,
            tc.tile_pool(name="s_ps", bufs=2, space=bass.MemorySpace.PSUM) as s_pool,
            tc.tile_pool(name="pv_ps", bufs=2, space=bass.MemorySpace.PSUM) as pv_pool,
            tc.tile_pool(name="mm_ps", bufs=2, space=bass.MemorySpace.PSUM) as mm_pool,
        ):
            # ---- persistent SBUF tensors (fp32r: matmul operands) ----
            qT_sb = pp.tile([TT, 2, S], F32R)      # [d%128, d//128, s]
            kT_sb = pp.tile([TT, 2, S], F32R)
            v_sb = pp.tile([TT, NTT, HPC, HD + 1], F32R)  # [t%128, t//128, h, d|1]
            wo_sb = pp.tile([TT, 2, HID], F32R)
            ones_sb = pp.tile([1, SC], F32R)
            mask_sb = pp.tile([TT, TT], F32R)
            bq_sb = pp.tile([1, DPC], F32R)
            bk_sb = pp.tile([1, DPC], F32R)
            zeros_sb = pp.tile([TT, 384], F32)
            # x half-buffer; weights stay resident across both halves
            x_sb = xp.tile([TT, NKT, S2], F32R)
            wq_sb = wp.tile([TT, NKT, DPC], F32R)
            wk_sb = wp.tile([TT, NKT, DPC], F32R)
            wv_sb = wp.tile([TT, NKT, DPC], F32R)

            # memset can't write fp32r; stage fp32 constants and round via DVE
            nc.vector.memset(zeros_sb[:], 0.0)
            ones_c = pp.tile([TT, NTT, HPC, 1], F32)
            nc.vector.memset(ones_c[:], 1.0)
            # ones columns of the augmented v (softmax denominator trick)
            nc.vector.tensor_copy(v_sb[:, :, :, HD : HD + 1], ones_c[:])

            def load_r(dst_ap, src_ap, shape, engine, q=None):
                stg = sp.tile([TT, S2], F32, tag="stg")
                s_ap = stg[: shape[0], : shape[1]]
                (q or nc.sync).dma_start(out=s_ap, in_=src_ap)
                if engine == "v":
                    nc.vector.tensor_copy(dst_ap, s_ap)
                else:
                    nc.scalar.activation(
                        dst_ap, s_ap, mybir.ActivationFunctionType.Copy
                    )

            def load_x(k, dst_c0, src_c0, width, engine):
                load_r(x_sb[:, k, dst_c0 : dst_c0 + width],
                       xT[128 * k : 128 * (k + 1), src_c0 : src_c0 + width],
                       (TT, width), engine,
                       q=(nc.sync if k % 2 == 0 else nc.scalar))

            # ---- projection / attention emission helpers ----
            # x_sb holds a sliding window of xT columns: phase A = t in
            # [0,1024); phase B overwrites cols [0,512) with t in [1024,1536);
            # phase C overwrites cols [512,1024) with t in [1536,2048).
            def proj_qk(w_sb, b_sb, dst, dti, sc, xoff):
                q_ps = mm_pool.tile([TT, SC], F32, tag="mm")
                for k in range(NKT):
                    nc.tensor.matmul(
                        q_ps[:],
                        w_sb[:, k, 128 * dti : 128 * (dti + 1)],
                        x_sb[:, k, xoff : xoff + SC],
                        start=(k == 0),
                        stop=(k == NKT - 1 and not has_bias),
                    )
                if has_bias:
                    nc.tensor.matmul(
                        q_ps[:],
                        b_sb[0:1, 128 * dti : 128 * (dti + 1)],
                        ones_sb[0:1, :],
                        start=False,
                        stop=True,
                    )
                nc.vector.tensor_copy(dst[:, dti, SC * sc : SC * (sc + 1)], q_ps[:])

            def proj_v(tt, xoff):
                v_ps = mm_pool.tile([TT, DPC], F32, tag="mm")
                for k in range(NKT):
                    nc.tensor.matmul(
                        v_ps[:],
                        x_sb[:, k, xoff : xoff + 128],
                        wv_sb[:, k, :],
                        start=(k == 0),
                        stop=(k == NKT - 1),
                    )
                nc.vector.tensor_copy(
                    v_sb[:, tt, :, 0:HD],
                    v_ps[:].rearrange("p (h d) -> p h d", h=HPC),
                )

            def attn_head(sc, h, attn_sb):
                dti, po = h // 2, 64 * (h % 2)
                n_tt = 4 * (sc + 1) if causal else NTT
                pv_ps = pv_pool.tile([HD + 1, SC], F32)
                for w0 in range(0, n_tt, WAVE):
                    wn = min(WAVE, n_tt - w0)
                    e_sb = e_pool.tile([TT, WAVE, SC], F32R)
                    # scoresT[t, s] blocks + exp (2 t-tiles per call;
                    # 2-bank groups x bufs=2 keep PE and ACT moving).
                    # Diagonal tiles r=1,2 compute only cols [128r:512]; the
                    # skipped region is zeroed below before PV reads it.
                    for g0 in range(0, wn, 2):
                        s_ps = s_pool.tile([TT, 2, SC], F32)
                        for i in range(2):
                            tt = w0 + g0 + i
                            nc.tensor.matmul(
                                s_ps[:, i, :],
                                kT_sb[po : po + 64, dti,
                                      128 * tt : 128 * (tt + 1)],
                                qT_sb[po : po + 64, dti,
                                      SC * sc : SC * (sc + 1)],
                                start=True,
                                stop=True,
                            )
                        nc.scalar.activation(
                            e_sb[:, g0 : g0 + 2, :],
                            s_ps[:],
                            EXP,
                            scale=float(1.0 / np.sqrt(HD)),
                        )
                    if causal and w0 + wn == n_tt:
                        # diagonal tiles: zero below-diagonal columns,
                        # triangular-mask the diagonal 128x128 block
                        for i in range(4):
                            wi = wn - 4 + i
                            c0 = 128 * i
                            if i > 0:
                                nc.vector.tensor_copy(
                                    e_sb[:, wi, 0:c0], zeros_sb[:, 0:c0]
                                )
                            nc.vector.tensor_mul(
                                e_sb[:, wi, c0 : c0 + 128],
                                e_sb[:, wi, c0 : c0 + 128],
                                mask_sb[:],
                            )
                    # PV: outT_aug[65, s] += v_aug[t, 65].T @ expT[t, s]
                    # (diagonal tiles r=1,2 skip their all-zero columns)
                    for wi in range(wn):
                        tt = w0 + wi
                        r = tt - (n_tt - 4) if causal else -1
                        c0 = 128 * r if r in (1, 2) else 0
                        nc.tensor.matmul(
                            pv_ps[:, c0:SC],
                            v_sb[:, tt, h, :],
                            e_sb[:, wi, c0:SC],
                            start=(tt == 0),
                            stop=(tt == n_tt - 1),
                        )
                # normalize: row 64 of pv_ps is the softmax denominator
                rc_sb = sm_pool.tile([1, SC], F32R, tag="rc")
                nc.vector.reciprocal(rc_sb[:], pv_ps[64:65, :])
                bc_ps = mm_pool.tile([HD, SC], F32, tag="mm")
                nc.tensor.matmul(
                    bc_ps[:],
                    ones_sb[0:1, 0:HD],
                    rc_sb[0:1, :],
                    start=True,
                    stop=True,
                )
                bc_sb = sm_pool.tile([HD, SC], F32, tag="bc")
                nc.vector.tensor_copy(bc_sb[:], bc_ps[:])
                nc.vector.tensor_mul(
                    attn_sb[po : po + 64, dti, :], pv_ps[0:64, :], bc_sb[:]
                )

            def attn_oproj(sc, attn_sb):
                for et in range(NKT):
                    o_ps = mm_pool.tile([TT, SC], F32, tag="mm")
                    for dti in range(2):
                        nc.tensor.matmul(
                            o_ps[:],
                            wo_sb[:, dti, 128 * et : 128 * (et + 1)],
                            attn_sb[:, dti, :],
                            start=(dti == 0),
                            stop=(dti == 1),
                        )
                    o_sb = o_pool.tile([TT, SC], F32)
                    nc.vector.tensor_copy(o_sb[:], o_ps[:])
                    nc.sync.dma_start(
                        out=outT[128 * et : 128 * (et + 1),
                                 SC * sc : SC * (sc + 1)],
                        in_=o_sb[:],
                    )

            # ---- stage A: constants, x cols [0,1024) + weights, then
            # projections for s-chunks 0-1 and t-tiles 0-7 ----
            ones_st = sp.tile([1, SC], F32, tag="ones_st")
            nc.vector.memset(ones_st[:], 1.0)
            nc.vector.tensor_copy(ones_sb[:], ones_st[:])
            load_r(mask_sb[:], mskd[:], (TT, TT), "v")
            if has_bias:
                load_r(bq_sb[:], bqr[:], (1, DPC), "v")
                load_r(bk_sb[:], bkr[:], (1, DPC), "v")
            for k in range(NKT):
                load_x(k, 0, 0, S2, "s")
                load_r(wq_sb[:, k, :], wqT[128 * k : 128 * (k + 1), :],
                       (TT, DPC), "v", q=nc.scalar)
                load_r(wk_sb[:, k, :], wkT[128 * k : 128 * (k + 1), :],
                       (TT, DPC), "v", q=nc.scalar)
            for k in range(NKT):
                load_r(wv_sb[:, k, :], wvT[128 * k : 128 * (k + 1), :],
                       (TT, DPC), "v", q=nc.scalar)
            for dti in range(2):
                load_r(wo_sb[:, dti, :], woT[128 * dti : 128 * (dti + 1), :],
                       (TT, HID), "v", q=nc.scalar)
            unitsA = []
            unitsA.append(lambda: proj_qk(wq_sb, bq_sb, qT_sb, 0, 0, 0))
            unitsA.append(lambda: proj_qk(wk_sb, bk_sb, kT_sb, 0, 0, 0))
            unitsA.append(lambda: proj_qk(wq_sb, bq_sb, qT_sb, 1, 0, 0))
            unitsA.append(lambda: proj_qk(wk_sb, bk_sb, kT_sb, 1, 0, 0))
            for i in range(4):
                unitsA.append(lambda i=i: proj_v(i, 128 * i))
            unitsA.append(lambda: proj_qk(wq_sb, bq_sb, qT_sb, 0, 1, SC))
            unitsA.append(lambda: proj_qk(wk_sb, bk_sb, kT_sb, 0, 1, SC))
            unitsA.append(lambda: proj_qk(wq_sb, bq_sb, qT_sb, 1, 1, SC))
            unitsA.append(lambda: proj_qk(wk_sb, bk_sb, kT_sb, 1, 1, SC))
            for i in range(4, 8):
                unitsA.append(lambda i=i: proj_v(i, 128 * i))
            for u in unitsA:
                u()

            # ---- stages B/C: x cols for t in [1024,2048) stream in while
            # attention runs; remaining projections interleave per head ----
            for k in range(NKT):
                load_x(k, 0, S2, SC, "s")        # phase B: t in [1024,1536)
            units = []
            units.append(lambda: proj_qk(wq_sb, bq_sb, qT_sb, 0, 2, 0))
            units.append(lambda: proj_qk(wk_sb, bk_sb, kT_sb, 0, 2, 0))
            units.append(lambda: proj_qk(wq_sb, bq_sb, qT_sb, 1, 2, 0))
            units.append(lambda: proj_qk(wk_sb, bk_sb, kT_sb, 1, 2, 0))
            for i in range(4):
                units.append(lambda i=i: proj_v(8 + i, 128 * i))
            units.append(lambda: proj_qk(wq_sb, bq_sb, qT_sb, 0, 3, SC))
            units.append(lambda: proj_qk(wk_sb, bk_sb, kT_sb, 0, 3, SC))
            units.append(lambda: proj_qk(wq_sb, bq_sb, qT_sb, 1, 3, SC))
            units.append(lambda: proj_qk(wk_sb, bk_sb, kT_sb, 1, 3, SC))
            for i in range(4):
                units.append(lambda i=i: proj_v(12 + i, SC + 128 * i))
            ui = 0
            for sc in range(NSC):
                attn_sb = attn_pool.tile([TT, 2, SC], F32R)
                for h in range(HPC):
                    attn_head(sc, h, attn_sb)
                    # 1 unit/head over sc 0-1 (deps for sc2), 2/head at sc2
                    for _ in range(1 if sc < 2 else 2):
                        if ui < len(units):
                            units[ui]()
                            ui += 1
                if sc == 0:
                    # phase C x loads: WAR on phase-A readers resolved by now
                    for k in range(NKT):
                        load_x(k, SC, S2 + SC, SC, "s")
                attn_oproj(sc, attn_sb)
            while ui < len(units):
                units[ui]()
                ui += 1
    ctx_lp.__exit__(None, None, None)
    nc.compile()
    return nc


_CACHE = {}
LAST_RESULTS = None


def _get_nc(causal: bool, has_bias: bool = False):
    key = (causal, has_bias)
    if key not in _CACHE:
        _CACHE[key] = _build(causal, has_bias)
    return _CACHE[key]


def _reference_host(hidden_state, attention_mask, wq, bq, wk, bk, wv, bv, wo, bo):
    """Exact numpy fallback for unexpected mask patterns."""
    x = hidden_state.astype(np.float64)
    q = (x @ wq.T.astype(np.float64) + bq).reshape(B, S, NH, HD).transpose(0, 2, 1, 3)
    k = (x @ wk.T.astype(np.float64) + bk).reshape(B, S, NH, HD).transpose(0, 2, 1, 3)
    v = (x @ wv.T.astype(np.float64) + bv).reshape(B, S, NH, HD).transpose(0, 2, 1, 3)
    sc = np.einsum("bhsd,bhtd->bhst", q, k) / np.sqrt(HD)
    sc = np.where(attention_mask, sc, -np.inf)
    sc -= sc.max(axis=-1, keepdims=True)
    e = np.exp(sc)
    p = e / e.sum(axis=-1, keepdims=True)
    o = np.einsum("bhst,bhtd->bhsd", p, v).transpose(0, 2, 1, 3).reshape(B, S, HID)
    return (o @ wo.T.astype(np.float64) + bo).astype(np.float32)


def kernel(hidden_state, attention_mask, wq, bq, wk, bk, wv, bv, wo, bo):
    global LAST_RESULTS
    hidden_state = np.asarray(hidden_state, dtype=np.float32)
    attention_mask = np.asarray(attention_mask, dtype=bool)
    wq, bq = np.asarray(wq, np.float32), np.asarray(bq, np.float32)
    wk, bk = np.asarray(wk, np.float32), np.asarray(bk, np.float32)
    wv, bv = np.asarray(wv, np.float32), np.asarray(bv, np.float32)
    wo, bo = np.asarray(wo, np.float32), np.asarray(bo, np.float32)

    tril = np.tril(np.ones((S, S), dtype=bool))
    if (attention_mask == tril).all():
        causal = True
    elif attention_mask.all():
        causal = False
    else:
        return _reference_host(
            hidden_state, attention_mask, wq, bq, wk, bk, wv, bv, wo, bo
        )

    mask_tri = np.triu(np.ones((TT, TT), dtype=np.float32))
    in_maps = []
    for c in range(N_CORES):
        b, g = c // 4, c % 4
        r0 = DPC * g
        in_maps.append(
            {
                "xT": np.ascontiguousarray(hidden_state[b].T),
                "wqT": np.ascontiguousarray(wq[r0 : r0 + DPC].T),
                "wkT": np.ascontiguousarray(wk[r0 : r0 + DPC].T),
                "wvT": np.ascontiguousarray(wv[r0 : r0 + DPC].T),
                "woT": np.ascontiguousarray(wo[:, r0 : r0 + DPC].T),
                "bq_r": np.ascontiguousarray(bq[r0 : r0 + DPC].reshape(1, DPC)),
                "bk_r": np.ascontiguousarray(bk[r0 : r0 + DPC].reshape(1, DPC)),
                "mask_tri": mask_tri,
            }
        )

    has_bias = bool(np.any(bq) or np.any(bk))
    nc = _get_nc(causal, has_bias)
    res = run_bass_kernel_spmd(nc, in_maps, list(range(N_CORES)))
    LAST_RESULTS = res

    out = np.zeros((B, S, HID), dtype=np.float32)
    for c in range(N_CORES):
        out[c // 4] += res.results[c]["outT"].T
    out += (bv @ wo.T + bo)[None, None, :]
    return out


# revision 34
# speedup vs baseline: 1.0130x; 1.0130x over previous
"""Trainium2 Bass kernel: 16-head causal MHA (B=2, S=2048, hidden=1024).

Sharding (data + head parallel over 8 cores): core c handles batch c//4
and heads [4*(c%4), 4*(c%4)+4). Each core computes its q/k/v projections,
causal attention for its 4 heads, and a partial o-projection restricted to
its head columns. The host sums the 4 partials per batch (the post-o_proj
all-reduce, done host-side during gather) and adds the exactly-linear bias
terms (bv @ wo.T + bo). bq/bk are applied on device via rank-1 bias
matmuls.

Layout strategy avoids all on-device transposes:
  - host passes hidden pre-transposed xT (HID, S) so projections produce
    qT/kT [d, s] directly and v in [t, d];
  - scores are computed transposed, scoresT[t, s] = kT-slice.T @ qT-slice,
    so the softmax-normalization sums over t arrive for free by augmenting
    v with a ones column in the PV matmul (row 64 of the PV output is the
    softmax denominator);
  - the per-column reciprocal is broadcast across partitions with a K=1
    matmul against a ones vector.

All matmuls run as float32r (full PE rate at free-dim >= 256, ~2x fp32
matmul throughput, ~16-bit-mantissa precision). The BIR verifier requires
every fp32r matmul operand to be produced by a compute op that rounds to
fp32r, so DMA loads land in a staging tile and are converted by DVE/ACT
copies. Softmax skips the max-subtraction: at this problem's scale the
scores are O(1) so exp is safe in fp32, and exp(s)*mask/sum equals
softmax(where(mask, s, -inf)) exactly.
"""

import numpy as np

import concourse.bass as bass
import concourse.mybir as mybir
import concourse.tile as tile
from concourse import bacc
from concourse.bass_utils import run_bass_kernel_spmd

B, S, HID = 2, 2048, 1024
NH, HD = 16, 64
N_CORES = 8
HPC = 4            # heads per core
DPC = HPC * HD     # 256 head-dims per core
SC = 512           # s-chunk (matmul free dim)
NSC = S // SC      # 4
TT = 128           # t-tile (partitions)
NTT = S // TT      # 16
NKT = HID // 128   # 8 contraction tiles for the projections

F32 = mybir.dt.float32
F32R = mybir.dt.float32r
EXP = mybir.ActivationFunctionType.Exp


def _build(causal: bool, has_bias: bool = True):
    nc = bacc.Bacc(
        "TRN2",
        target_bir_lowering=False,
        debug=False,
        enable_asserts=False,
        num_devices=N_CORES,
    )
    xT = nc.dram_tensor("xT", [HID, S], F32, kind="ExternalInput").ap()
    wqT = nc.dram_tensor("wqT", [HID, DPC], F32, kind="ExternalInput").ap()
    wkT = nc.dram_tensor("wkT", [HID, DPC], F32, kind="ExternalInput").ap()
    wvT = nc.dram_tensor("wvT", [HID, DPC], F32, kind="ExternalInput").ap()
    woT = nc.dram_tensor("woT", [DPC, HID], F32, kind="ExternalInput").ap()
    bqr = nc.dram_tensor("bq_r", [1, DPC], F32, kind="ExternalInput").ap()
    bkr = nc.dram_tensor("bk_r", [1, DPC], F32, kind="ExternalInput").ap()
    mskd = nc.dram_tensor("mask_tri", [TT, TT], F32, kind="ExternalInput").ap()
    outT = nc.dram_tensor("outT", [HID, S], F32, kind="ExternalOutput").ap()

    S2 = S // 2          # 1024: columns per half
    NS2 = NSC // 2       # 2 s-chunks per half
    NT2 = NTT // 2       # 8 t-tiles per half
    WAVE = 8             # t-tiles per exp wave

    ctx_lp = nc.allow_low_precision(reason="fp32r matmul pipeline (deliberate)")
    ctx_lp.__enter__()
    with tile.TileContext(nc) as tc:
        with (
            tc.tile_pool(name="persist", bufs=1) as pp,
            tc.tile_pool(name="xpool", bufs=1) as xp,
            tc.tile_pool(name="wpool", bufs=1) as wp,
            tc.tile_pool(name="stage", bufs=3) as sp,
            tc.tile_pool(name="expbuf", bufs=2) as e_pool,
            tc.tile_pool(name="attn", bufs=2) as attn_pool,
            tc.tile_pool(name="osb", bufs=3) as o_pool,
            tc.tile_pool(name="small", bufs=2) as sm_pool,
            tc.tile_pool(name="s_ps", bufs=2, space=bass.MemorySpace.PSUM) as s_pool,
            tc.tile_pool(name="pv_ps", bufs=2, space=bass.MemorySpace.PSUM) as pv_pool,
            tc.tile_pool(name="mm_ps", bufs=2, space=bass.MemorySpace.PSUM) as mm_pool,
        ):
            # ---- persistent SBUF tensors (fp32r: matmul operands) ----
            qT_sb = pp.tile([TT, 2, S], F32R)      # [d%128, d//128, s]
            kT_sb = pp.tile([TT, 2, S], F32R)
            v_sb = pp.tile([TT, NTT, HPC, HD + 1], F32R)  # [t%128, t//128, h, d|1]
            wo_sb = pp.tile([TT, 2, HID], F32R)
            ones_sb = pp.tile([1, SC], F32R)
            mask_sb = pp.tile([TT, TT], F32R)
            bq_sb = pp.tile([1, DPC], F32R)
            bk_sb = pp.tile([1, DPC], F32R)
            zeros_sb = pp.tile([TT, 384], F32)
            # x half-buffer; weights stay resident across both halves
            x_sb = xp.tile([TT, NKT, S2], F32R)
            wq_sb = wp.tile([TT, NKT, DPC], F32R)
            wk_sb = wp.tile([TT, NKT, DPC], F32R)
            wv_sb = wp.tile([TT, NKT, DPC], F32R)

            # memset can't write fp32r; stage fp32 constants and round via DVE
            nc.vector.memset(zeros_sb[:], 0.0)
            ones_c = pp.tile([TT, NTT, HPC, 1], F32)
            nc.vector.memset(ones_c[:], 1.0)
            # ones columns of the augmented v (softmax denominator trick)
            nc.vector.tensor_copy(v_sb[:, :, :, HD : HD + 1], ones_c[:])

            def load_r(dst_ap, src_ap, shape, engine, q=None):
                stg = sp.tile([TT, S2], F32, tag="stg")
                s_ap = stg[: shape[0], : shape[1]]
                (q or nc.sync).dma_start(out=s_ap, in_=src_ap)
                if engine == "v":
                    nc.vector.tensor_copy(dst_ap, s_ap)
                else:
                    nc.scalar.activation(
                        dst_ap, s_ap, mybir.ActivationFunctionType.Copy
                    )

            def load_x(k, dst_c0, src_c0, width, engine):
                load_r(x_sb[:, k, dst_c0 : dst_c0 + width],
                       xT[128 * k : 128 * (k + 1), src_c0 : src_c0 + width],
                       (TT, width), engine,
                       q=(nc.sync if k % 2 == 0 else nc.scalar))

            # ---- projection / attention emission helpers ----
            # x_sb holds a sliding window of xT columns: phase A = t in
            # [0,1024); phase B overwrites cols [0,512) with t in [1024,1536);
            # phase C overwrites cols [512,1024) with t in [1536,2048).
            def proj_qk(w_sb, b_sb, dst, dti, sc, xoff):
                q_ps = mm_pool.tile([TT, SC], F32, tag="mm")
                for k in range(NKT):
                    nc.tensor.matmul(
                        q_ps[:],
                        w_sb[:, k, 128 * dti : 128 * (dti + 1)],
                        x_sb[:, k, xoff : xoff + SC],
                        start=(k == 0),
                        stop=(k == NKT - 1 and not has_bias),
                    )
                if has_bias:
                    nc.tensor.matmul(
                        q_ps[:],
                        b_sb[0:1, 128 * dti : 128 * (dti + 1)],
                        ones_sb[0:1, :],
                        start=False,
                        stop=True,
                    )
                nc.vector.tensor_copy(dst[:, dti, SC * sc : SC * (sc + 1)], q_ps[:])

            def proj_v(tt, xoff):
                v_ps = mm_pool.tile([TT, DPC], F32, tag="mm")
                for k in range(NKT):
                    nc.tensor.matmul(
                        v_ps[:],
                        x_sb[:, k, xoff : xoff + 128],
                        wv_sb[:, k, :],
                        start=(k == 0),
                        stop=(k == NKT - 1),
                    )
                nc.vector.tensor_copy(
                    v_sb[:, tt, :, 0:HD],
                    v_ps[:].rearrange("p (h d) -> p h d", h=HPC),
                )

            def attn_head(sc, h, attn_sb):
                dti, po = h // 2, 64 * (h % 2)
                n_tt = 4 * (sc + 1) if causal else NTT
                pv_ps = pv_pool.tile([HD + 1, SC], F32)
                for w0 in range(0, n_tt, WAVE):
                    wn = min(WAVE, n_tt - w0)
                    e_sb = e_pool.tile([TT, WAVE, SC], F32R)
                    # scoresT[t, s] blocks + exp (2 t-tiles per call;
                    # 2-bank groups x bufs=2 keep PE and ACT moving).
                    # Diagonal tiles r=1,2 compute only cols [128r:512]; the
                    # skipped region is zeroed below before PV reads it.
                    for g0 in range(0, wn, 2):
                        s_ps = s_pool.tile([TT, 2, SC], F32)
                        for i in range(2):
                            tt = w0 + g0 + i
                            nc.tensor.matmul(
                                s_ps[:, i, :],
                                kT_sb[po : po + 64, dti,
                                      128 * tt : 128 * (tt + 1)],
                                qT_sb[po : po + 64, dti,
                                      SC * sc : SC * (sc + 1)],
                                start=True,
                                stop=True,
                            )
                        nc.scalar.activation(
                            e_sb[:, g0 : g0 + 2, :],
                            s_ps[:],
                            EXP,
                            scale=float(1.0 / np.sqrt(HD)),
                        )
                    if causal and w0 + wn == n_tt:
                        # diagonal tiles: zero below-diagonal columns,
                        # triangular-mask the diagonal 128x128 block
                        for i in range(4):
                            wi = wn - 4 + i
                            c0 = 128 * i
                            if i > 0:
                                nc.vector.tensor_copy(
                                    e_sb[:, wi, 0:c0], zeros_sb[:, 0:c0]
                                )
                            nc.vector.tensor_mul(
                                e_sb[:, wi, c0 : c0 + 128],
                                e_sb[:, wi, c0 : c0 + 128],
                                mask_sb[:],
                            )
                    # PV: outT_aug[65, s] += v_aug[t, 65].T @ expT[t, s]
                    # (diagonal tiles r=1,2 skip their all-zero columns)
                    for wi in range(wn):
                        tt = w0 + wi
                        r = tt - (n_tt - 4) if causal else -1
                        c0 = 128 * r if r in (1, 2) else 0
                        nc.tensor.matmul(
                            pv_ps[:, c0:SC],
                            v_sb[:, tt, h, :],
                            e_sb[:, wi, c0:SC],
                            start=(tt == 0),
                            stop=(tt == n_tt - 1),
                        )
                # normalize: row 64 of pv_ps is the softmax denominator
                rc_sb = sm_pool.tile([1, SC], F32R, tag="rc")
                nc.vector.reciprocal(rc_sb[:], pv_ps[64:65, :])
                bc_ps = mm_pool.tile([HD, SC], F32, tag="mm")
                nc.tensor.matmul(
                    bc_ps[:],
                    ones_sb[0:1, 0:HD],
                    rc_sb[0:1, :],
                    start=True,
                    stop=True,
                )
                bc_sb = sm_pool.tile([HD, SC], F32, tag="bc")
                nc.vector.tensor_copy(bc_sb[:], bc_ps[:])
                nc.vector.tensor_mul(
                    attn_sb[po : po + 64, dti, :], pv_ps[0:64, :], bc_sb[:]
                )

            def attn_oproj(sc, attn_sb):
                for et in range(NKT):
                    o_ps = mm_pool.tile([TT, SC], F32, tag="mm")
                    for dti in range(2):
                        nc.tensor.matmul(
                            o_ps[:],
                            wo_sb[:, dti, 128 * et : 128 * (et + 1)],
                            attn_sb[:, dti, :],
                            start=(dti == 0),
                            stop=(dti == 1),
                        )
                    o_sb = o_pool.tile([TT, SC], F32)
                    nc.vector.tensor_copy(o_sb[:], o_ps[:])
                    nc.sync.dma_start(
                        out=outT[128 * et : 128 * (et + 1),
                                 SC * sc : SC * (sc + 1)],
                        in_=o_sb[:],
                    )

            # ---- stage A: constants, x cols [0,1024) + weights, then
            # projections for s-chunks 0-1 and t-tiles 0-7 ----
            ones_st = sp.tile([1, SC], F32, tag="ones_st")
            nc.vector.memset(ones_st[:], 1.0)
            nc.vector.tensor_copy(ones_sb[:], ones_st[:])
            load_r(mask_sb[:], mskd[:], (TT, TT), "v")
            if has_bias:
                load_r(bq_sb[:], bqr[:], (1, DPC), "v")
                load_r(bk_sb[:], bkr[:], (1, DPC), "v")
            for k in range(NKT):
                load_x(k, 0, 0, S2, "s")
                load_r(wq_sb[:, k, :], wqT[128 * k : 128 * (k + 1), :],
                       (TT, DPC), "v", q=nc.scalar)
                load_r(wk_sb[:, k, :], wkT[128 * k : 128 * (k + 1), :],
                       (TT, DPC), "v", q=nc.scalar)
            for k in range(NKT):
                load_r(wv_sb[:, k, :], wvT[128 * k : 128 * (k + 1), :],
                       (TT, DPC), "v", q=nc.scalar)
            for dti in range(2):
                load_r(wo_sb[:, dti, :], woT[128 * dti : 128 * (dti + 1), :],
                       (TT, HID), "v", q=nc.scalar)
            unitsA = []
            unitsA.append(lambda: proj_qk(wq_sb, bq_sb, qT_sb, 0, 0, 0))
            unitsA.append(lambda: proj_qk(wk_sb, bk_sb, kT_sb, 0, 0, 0))
            unitsA.append(lambda: proj_qk(wq_sb, bq_sb, qT_sb, 1, 0, 0))
            unitsA.append(lambda: proj_qk(wk_sb, bk_sb, kT_sb, 1, 0, 0))
            for i in range(4):
                unitsA.append(lambda i=i: proj_v(i, 128 * i))
            unitsA.append(lambda: proj_qk(wq_sb, bq_sb, qT_sb, 0, 1, SC))
            unitsA.append(lambda: proj_qk(wk_sb, bk_sb, kT_sb, 0, 1, SC))
            unitsA.append(lambda: proj_qk(wq_sb, bq_sb, qT_sb, 1, 1, SC))
            unitsA.append(lambda: proj_qk(wk_sb, bk_sb, kT_sb, 1, 1, SC))
            for i in range(4, 8):
                unitsA.append(lambda i=i: proj_v(i, 128 * i))
            for u in unitsA:
                u()

            # ---- stages B/C: x cols for t in [1024,2048) stream in while
            # attention runs; remaining projections interleave per head ----
            for k in range(NKT):
                load_x(k, 0, S2, SC, "v")        # phase B: t in [1024,1536)
            units = []
            units.append(lambda: proj_qk(wq_sb, bq_sb, qT_sb, 0, 2, 0))
            units.append(lambda: proj_qk(wk_sb, bk_sb, kT_sb, 0, 2, 0))
            units.append(lambda: proj_qk(wq_sb, bq_sb, qT_sb, 1, 2, 0))
            units.append(lambda: proj_qk(wk_sb, bk_sb, kT_sb, 1, 2, 0))
            for i in range(4):
                units.append(lambda i=i: proj_v(8 + i, 128 * i))
            units.append(lambda: proj_qk(wq_sb, bq_sb, qT_sb, 0, 3, SC))
            units.append(lambda: proj_qk(wk_sb, bk_sb, kT_sb, 0, 3, SC))
            units.append(lambda: proj_qk(wq_sb, bq_sb, qT_sb, 1, 3, SC))
            units.append(lambda: proj_qk(wk_sb, bk_sb, kT_sb, 1, 3, SC))
            for i in range(4):
                units.append(lambda i=i: proj_v(12 + i, SC + 128 * i))
            ui = 0
            for sc in range(NSC):
                attn_sb = attn_pool.tile([TT, 2, SC], F32R)
                for h in range(HPC):
                    attn_head(sc, h, attn_sb)
                    # 1 unit/head over sc 0-1 (deps for sc2), 2/head at sc2
                    for _ in range(1 if sc < 2 else 2):
                        if ui < len(units):
                            units[ui]()
                            ui += 1
                if sc == 0:
                    # phase C x loads: WAR on phase-A readers resolved by now
                    for k in range(NKT):
                        load_x(k, SC, S2 + SC, SC, "v")
                attn_oproj(sc, attn_sb)
            while ui < len(units):
                units[ui]()
                ui += 1
    ctx_lp.__exit__(None, None, None)
    nc.compile()
    return nc


_CACHE = {}
LAST_RESULTS = None


def _get_nc(causal: bool, has_bias: bool = False):
    key = (causal, has_bias)
    if key not in _CACHE:
        _CACHE[key] = _build(causal, has_bias)
    return _CACHE[key]


def _reference_host(hidden_state, attention_mask, wq, bq, wk, bk, wv, bv, wo, bo):
    """Exact numpy fallback for unexpected mask patterns."""
    x = hidden_state.astype(np.float64)
    q = (x @ wq.T.astype(np.float64) + bq).reshape(B, S, NH, HD).transpose(0, 2, 1, 3)
    k = (x @ wk.T.astype(np.float64) + bk).reshape(B, S, NH, HD).transpose(0, 2, 1, 3)
    v = (x @ wv.T.astype(np.float64) + bv).reshape(B, S, NH, HD).transpose(0, 2, 1, 3)
    sc = np.einsum("bhsd,bhtd->bhst", q, k) / np.sqrt(HD)
    sc = np.where(attention_mask, sc, -np.inf)
    sc -= sc.max(axis=-1, keepdims=True)
    e = np.exp(sc)
    p = e / e.sum(axis=-1, keepdims=True)
    o = np.einsum("bhst,bhtd->bhsd", p, v).transpose(0, 2, 1, 3).reshape(B, S, HID)
    return (o @ wo.T.astype(np.float64) + bo).astype(np.float32)


def kernel(hidden_state, attention_mask, wq, bq, wk, bk, wv, bv, wo, bo):
    global LAST_RESULTS
    hidden_state = np.asarray(hidden_state, dtype=np.float32)
    attention_mask = np.asarray(attention_mask, dtype=bool)
    wq, bq = np.asarray(wq, np.float32), np.asarray(bq, np.float32)
    wk, bk = np.asarray(wk, np.float32), np.asarray(bk, np.float32)
    wv, bv = np.asarray(wv, np.float32), np.asarray(bv, np.float32)
    wo, bo = np.asarray(wo, np.float32), np.asarray(bo, np.float32)

    tril = np.tril(np.ones((S, S), dtype=bool))
    if (attention_mask == tril).all():
        causal = True
    elif attention_mask.all():
        causal = False
    else:
        return _reference_host(
            hidden_state, attention_mask, wq, bq, wk, bk, wv, bv, wo, bo
        )

    mask_tri = np.triu(np.ones((TT, TT), dtype=np.float32))
    in_maps = []
    for c in range(N_CORES):
        b, g = c // 4, c % 4
        r0 = DPC * g
        in_maps.append(
            {
                "xT": np.ascontiguousarray(hidden_state[b].T),
                "wqT": np.ascontiguousarray(wq[r0 : r0 + DPC].T),
                "wkT": np.ascontiguousarray(wk[r0 : r0 + DPC].T),
                "wvT": np.ascontiguousarray(wv[r0 : r0 + DPC].T),
                "woT": np.ascontiguousarray(wo[:, r0 : r0 + DPC].T),
                "bq_r": np.ascontiguousarray(bq[r0 : r0 + DPC].reshape(1, DPC)),
                "bk_r": np.ascontiguousarray(bk[r0 : r0 + DPC].reshape(1, DPC)),
                "mask_tri": mask_tri,
            }
        )

    has_bias = bool(np.any(bq) or np.any(bk))
    nc = _get_nc(causal, has_bias)
    res = run_bass_kernel_spmd(nc, in_maps, list(range(N_CORES)))
    LAST_RESULTS = res

    out = np.zeros((B, S, HID), dtype=np.float32)
    for c in range(N_CORES):
        out[c // 4] += res.results[c]["outT"].T
    out += (bv @ wo.T + bo)[None, None, :]
    return out


# revision 35
# speedup vs baseline: 1.0137x; 1.0006x over previous
"""Trainium2 Bass kernel: 16-head causal MHA (B=2, S=2048, hidden=1024).

Sharding (data + head parallel over 8 cores): core c handles batch c//4
and heads [4*(c%4), 4*(c%4)+4). Each core computes its q/k/v projections,
causal attention for its 4 heads, and a partial o-projection restricted to
its head columns. The host sums the 4 partials per batch (the post-o_proj
all-reduce, done host-side during gather) and adds the exactly-linear bias
terms (bv @ wo.T + bo). bq/bk are applied on device via rank-1 bias
matmuls.

Layout strategy avoids all on-device transposes:
  - host passes hidden pre-transposed xT (HID, S) so projections produce
    qT/kT [d, s] directly and v in [t, d];
  - scores are computed transposed, scoresT[t, s] = kT-slice.T @ qT-slice,
    so the softmax-normalization sums over t arrive for free by augmenting
    v with a ones column in the PV matmul (row 64 of the PV output is the
    softmax denominator);
  - the per-column reciprocal is broadcast across partitions with a K=1
    matmul against a ones vector.

All matmuls run as float32r (full PE rate at free-dim >= 256, ~2x fp32
matmul throughput, ~16-bit-mantissa precision). The BIR verifier requires
every fp32r matmul operand to be produced by a compute op that rounds to
fp32r, so DMA loads land in a staging tile and are converted by DVE/ACT
copies. Softmax skips the max-subtraction: at this problem's scale the
scores are O(1) so exp is safe in fp32, and exp(s)*mask/sum equals
softmax(where(mask, s, -inf)) exactly.
"""

import numpy as np

import concourse.bass as bass
import concourse.mybir as mybir
import concourse.tile as tile
from concourse import bacc
from concourse.bass_utils import run_bass_kernel_spmd

B, S, HID = 2, 2048, 1024
NH, HD = 16, 64
N_CORES = 8
HPC = 4            # heads per core
DPC = HPC * HD     # 256 head-dims per core
SC = 512           # s-chunk (matmul free dim)
NSC = S // SC      # 4
TT = 128           # t-tile (partitions)
NTT = S // TT      # 16
NKT = HID // 128   # 8 contraction tiles for the projections

F32 = mybir.dt.float32
F32R = mybir.dt.float32r
EXP = mybir.ActivationFunctionType.Exp


def _build(causal: bool, has_bias: bool = True):
    nc = bacc.Bacc(
        "TRN2",
        target_bir_lowering=False,
        debug=False,
        enable_asserts=False,
        num_devices=N_CORES,
    )
    xT = nc.dram_tensor("xT", [HID, S], F32, kind="ExternalInput").ap()
    wqT = nc.dram_tensor("wqT", [HID, DPC], F32, kind="ExternalInput").ap()
    wkT = nc.dram_tensor("wkT", [HID, DPC], F32, kind="ExternalInput").ap()
    wvT = nc.dram_tensor("wvT", [HID, DPC], F32, kind="ExternalInput").ap()
    woT = nc.dram_tensor("woT", [DPC, HID], F32, kind="ExternalInput").ap()
    bqr = nc.dram_tensor("bq_r", [1, DPC], F32, kind="ExternalInput").ap()
    bkr = nc.dram_tensor("bk_r", [1, DPC], F32, kind="ExternalInput").ap()
    mskd = nc.dram_tensor("mask_tri", [TT, TT], F32, kind="ExternalInput").ap()
    outT = nc.dram_tensor("outT", [HID, S], F32, kind="ExternalOutput").ap()

    S2 = S // 2          # 1024: columns per half
    NS2 = NSC // 2       # 2 s-chunks per half
    NT2 = NTT // 2       # 8 t-tiles per half
    WAVE = 8             # t-tiles per exp wave

    ctx_lp = nc.allow_low_precision(reason="fp32r matmul pipeline (deliberate)")
    ctx_lp.__enter__()
    with tile.TileContext(nc) as tc:
        with (
            tc.tile_pool(name="persist", bufs=1) as pp,
            tc.tile_pool(name="xpool", bufs=1) as xp,
            tc.tile_pool(name="wpool", bufs=1) as wp,
            tc.tile_pool(name="stage", bufs=3) as sp,
            tc.tile_pool(name="expbuf", bufs=2) as e_pool,
            tc.tile_pool(name="attn", bufs=2) as attn_pool,
            tc.tile_pool(name="osb", bufs=3) as o_pool,
            tc.tile_pool(name="small", bufs=2) as sm_pool,
            tc.tile_pool(name="s_ps", bufs=2, space=bass.MemorySpace.PSUM) as s_pool,
            tc.tile_pool(name="pv_ps", bufs=2, space=bass.MemorySpace.PSUM) as pv_pool,
            tc.tile_pool(name="mm_ps", bufs=2, space=bass.MemorySpace.PSUM) as mm_pool,
        ):
            # ---- persistent SBUF tensors (fp32r: matmul operands) ----
            qT_sb = pp.tile([TT, 2, S], F32R)      # [d%128, d//128, s]
            kT_sb = pp.tile([TT, 2, S], F32R)
            v_sb = pp.tile([TT, NTT, HPC, HD + 1], F32R)  # [t%128, t//128, h, d|1]
            wo_sb = pp.tile([TT, 2, HID], F32R)
            ones_sb = pp.tile([1, SC], F32R)
            mask_sb = pp.tile([TT, TT], F32R)
            bq_sb = pp.tile([1, DPC], F32R)
            bk_sb = pp.tile([1, DPC], F32R)
            zeros_sb = pp.tile([TT, 384], F32)
            # x half-buffer; weights stay resident across both halves
            x_sb = xp.tile([TT, NKT, S2], F32R)
            wq_sb = wp.tile([TT, NKT, DPC], F32R)
            wk_sb = wp.tile([TT, NKT, DPC], F32R)
            wv_sb = wp.tile([TT, NKT, DPC], F32R)

            # memset can't write fp32r; stage fp32 constants and round via DVE
            nc.vector.memset(zeros_sb[:], 0.0)
            ones_c = pp.tile([TT, NTT, HPC, 1], F32)
            nc.vector.memset(ones_c[:], 1.0)
            # ones columns of the augmented v (softmax denominator trick)
            nc.vector.tensor_copy(v_sb[:, :, :, HD : HD + 1], ones_c[:])

            def load_r(dst_ap, src_ap, shape, engine, q=None):
                stg = sp.tile([TT, S2], F32, tag="stg")
                s_ap = stg[: shape[0], : shape[1]]
                (q or nc.sync).dma_start(out=s_ap, in_=src_ap)
                if engine == "v":
                    nc.vector.tensor_copy(dst_ap, s_ap)
                else:
                    nc.scalar.activation(
                        dst_ap, s_ap, mybir.ActivationFunctionType.Copy
                    )

            def load_x(k, dst_c0, src_c0, width, engine):
                load_r(x_sb[:, k, dst_c0 : dst_c0 + width],
                       xT[128 * k : 128 * (k + 1), src_c0 : src_c0 + width],
                       (TT, width), engine,
                       q=(nc.sync if k % 2 == 0 else nc.scalar))

            # ---- projection / attention emission helpers ----
            # x_sb holds a sliding window of xT columns: phase A = t in
            # [0,1024); phase B overwrites cols [0,512) with t in [1024,1536);
            # phase C overwrites cols [512,1024) with t in [1536,2048).
            def proj_qk(w_sb, b_sb, dst, dti, sc, xoff):
                q_ps = mm_pool.tile([TT, SC], F32, tag="mm")
                for k in range(NKT):
                    nc.tensor.matmul(
                        q_ps[:],
                        w_sb[:, k, 128 * dti : 128 * (dti + 1)],
                        x_sb[:, k, xoff : xoff + SC],
                        start=(k == 0),
                        stop=(k == NKT - 1 and not has_bias),
                    )
                if has_bias:
                    nc.tensor.matmul(
                        q_ps[:],
                        b_sb[0:1, 128 * dti : 128 * (dti + 1)],
                        ones_sb[0:1, :],
                        start=False,
                        stop=True,
                    )
                nc.vector.tensor_copy(dst[:, dti, SC * sc : SC * (sc + 1)], q_ps[:])

            def proj_v(tt, xoff):
                v_ps = mm_pool.tile([TT, DPC], F32, tag="mm")
                for k in range(NKT):
                    nc.tensor.matmul(
                        v_ps[:],
                        x_sb[:, k, xoff : xoff + 128],
                        wv_sb[:, k, :],
                        start=(k == 0),
                        stop=(k == NKT - 1),
                    )
                nc.vector.tensor_copy(
                    v_sb[:, tt, :, 0:HD],
                    v_ps[:].rearrange("p (h d) -> p h d", h=HPC),
                )

            def attn_head(sc, h, attn_sb):
                dti, po = h // 2, 64 * (h % 2)
                n_tt = 4 * (sc + 1) if causal else NTT
                pv_ps = pv_pool.tile([HD + 1, SC], F32)
                for w0 in range(0, n_tt, WAVE):
                    wn = min(WAVE, n_tt - w0)
                    e_sb = e_pool.tile([TT, WAVE, SC], F32R)
                    # scoresT[t, s] blocks + exp (2 t-tiles per call;
                    # 2-bank groups x bufs=2 keep PE and ACT moving).
                    # Diagonal tiles r=1,2 compute only cols [128r:512]; the
                    # skipped region is zeroed below before PV reads it.
                    for g0 in range(0, wn, 2):
                        s_ps = s_pool.tile([TT, 2, SC], F32)
                        for i in range(2):
                            tt = w0 + g0 + i
                            nc.tensor.matmul(
                                s_ps[:, i, :],
                                kT_sb[po : po + 64, dti,
                                      128 * tt : 128 * (tt + 1)],
                                qT_sb[po : po + 64, dti,
                                      SC * sc : SC * (sc + 1)],
                                start=True,
                                stop=True,
                            )
                        nc.scalar.activation(
                            e_sb[:, g0 : g0 + 2, :],
                            s_ps[:],
                            EXP,
                            scale=float(1.0 / np.sqrt(HD)),
                        )
                    if causal and w0 + wn == n_tt:
                        # diagonal tiles: zero below-diagonal columns,
                        # triangular-mask the diagonal 128x128 block
                        for i in range(4):
                            wi = wn - 4 + i
                            c0 = 128 * i
                            if i > 0:
                                nc.vector.tensor_copy(
                                    e_sb[:, wi, 0:c0], zeros_sb[:, 0:c0]
                                )
                            nc.vector.tensor_mul(
                                e_sb[:, wi, c0 : c0 + 128],
                                e_sb[:, wi, c0 : c0 + 128],
                                mask_sb[:],
                            )
                    # PV: outT_aug[65, s] += v_aug[t, 65].T @ expT[t, s]
                    # (diagonal tiles r=1,2 skip their all-zero columns)
                    for wi in range(wn):
                        tt = w0 + wi
                        r = tt - (n_tt - 4) if causal else -1
                        c0 = 128 * r if r in (1, 2) else 0
                        nc.tensor.matmul(
                            pv_ps[:, c0:SC],
                            v_sb[:, tt, h, :],
                            e_sb[:, wi, c0:SC],
                            start=(tt == 0),
                            stop=(tt == n_tt - 1),
                        )
                # normalize: row 64 of pv_ps is the softmax denominator
                rc_sb = sm_pool.tile([1, SC], F32R, tag="rc")
                nc.vector.reciprocal(rc_sb[:], pv_ps[64:65, :])
                bc_ps = mm_pool.tile([HD, SC], F32, tag="mm")
                nc.tensor.matmul(
                    bc_ps[:],
                    ones_sb[0:1, 0:HD],
                    rc_sb[0:1, :],
                    start=True,
                    stop=True,
                )
                bc_sb = sm_pool.tile([HD, SC], F32, tag="bc")
                nc.vector.tensor_copy(bc_sb[:], bc_ps[:])
                nc.vector.tensor_mul(
                    attn_sb[po : po + 64, dti, :], pv_ps[0:64, :], bc_sb[:]
                )

            def attn_oproj(sc, attn_sb):
                for et in range(NKT):
                    o_ps = mm_pool.tile([TT, SC], F32, tag="mm")
                    for dti in range(2):
                        nc.tensor.matmul(
                            o_ps[:],
                            wo_sb[:, dti, 128 * et : 128 * (et + 1)],
                            attn_sb[:, dti, :],
                            start=(dti == 0),
                            stop=(dti == 1),
                        )
                    o_sb = o_pool.tile([TT, SC], F32)
                    nc.vector.tensor_copy(o_sb[:], o_ps[:])
                    nc.sync.dma_start(
                        out=outT[128 * et : 128 * (et + 1),
                                 SC * sc : SC * (sc + 1)],
                        in_=o_sb[:],
                    )

            # ---- stage A: constants, x cols [0,1024) + weights, then
            # projections for s-chunks 0-1 and t-tiles 0-7 ----
            ones_st = sp.tile([1, SC], F32, tag="ones_st")
            nc.vector.memset(ones_st[:], 1.0)
            nc.vector.tensor_copy(ones_sb[:], ones_st[:])
            load_r(mask_sb[:], mskd[:], (TT, TT), "v")
            if has_bias:
                load_r(bq_sb[:], bqr[:], (1, DPC), "v")
                load_r(bk_sb[:], bkr[:], (1, DPC), "v")
            for k in range(NKT):
                load_x(k, 0, 0, S2, "s")
                load_r(wq_sb[:, k, :], wqT[128 * k : 128 * (k + 1), :],
                       (TT, DPC), "v", q=nc.scalar)
                load_r(wk_sb[:, k, :], wkT[128 * k : 128 * (k + 1), :],
                       (TT, DPC), "v", q=nc.sync)
            for k in range(NKT):
                load_r(wv_sb[:, k, :], wvT[128 * k : 128 * (k + 1), :],
                       (TT, DPC), "v", q=(nc.sync if k % 2 == 0 else nc.scalar))
            for dti in range(2):
                load_r(wo_sb[:, dti, :], woT[128 * dti : 128 * (dti + 1), :],
                       (TT, HID), "v", q=nc.scalar)
            unitsA = []
            unitsA.append(lambda: proj_qk(wq_sb, bq_sb, qT_sb, 0, 0, 0))
            unitsA.append(lambda: proj_qk(wk_sb, bk_sb, kT_sb, 0, 0, 0))
            unitsA.append(lambda: proj_qk(wq_sb, bq_sb, qT_sb, 1, 0, 0))
            unitsA.append(lambda: proj_qk(wk_sb, bk_sb, kT_sb, 1, 0, 0))
            for i in range(4):
                unitsA.append(lambda i=i: proj_v(i, 128 * i))
            unitsA.append(lambda: proj_qk(wq_sb, bq_sb, qT_sb, 0, 1, SC))
            unitsA.append(lambda: proj_qk(wk_sb, bk_sb, kT_sb, 0, 1, SC))
            unitsA.append(lambda: proj_qk(wq_sb, bq_sb, qT_sb, 1, 1, SC))
            unitsA.append(lambda: proj_qk(wk_sb, bk_sb, kT_sb, 1, 1, SC))
            for i in range(4, 8):
                unitsA.append(lambda i=i: proj_v(i, 128 * i))
            for u in unitsA:
                u()

            # ---- stages B/C: x cols for t in [1024,2048) stream in while
            # attention runs; remaining projections interleave per head ----
            for k in range(NKT):
                load_x(k, 0, S2, SC, "v")        # phase B: t in [1024,1536)
            units = []
            units.append(lambda: proj_qk(wq_sb, bq_sb, qT_sb, 0, 2, 0))
            units.append(lambda: proj_qk(wk_sb, bk_sb, kT_sb, 0, 2, 0))
            units.append(lambda: proj_qk(wq_sb, bq_sb, qT_sb, 1, 2, 0))
            units.append(lambda: proj_qk(wk_sb, bk_sb, kT_sb, 1, 2, 0))
            for i in range(4):
                units.append(lambda i=i: proj_v(8 + i, 128 * i))
            units.append(lambda: proj_qk(wq_sb, bq_sb, qT_sb, 0, 3, SC))
            units.append(lambda: proj_qk(wk_sb, bk_sb, kT_sb, 0, 3, SC))
            units.append(lambda: proj_qk(wq_sb, bq_sb, qT_sb, 1, 3, SC))
            units.append(lambda: proj_qk(wk_sb, bk_sb, kT_sb, 1, 3, SC))
            for i in range(4):
                units.append(lambda i=i: proj_v(12 + i, SC + 128 * i))
            ui = 0
            for sc in range(NSC):
                attn_sb = attn_pool.tile([TT, 2, SC], F32R)
                for h in range(HPC):
                    attn_head(sc, h, attn_sb)
                    # 1 unit/head over sc 0-1 (deps for sc2), 2/head at sc2
                    for _ in range(1 if sc < 2 else 2):
                        if ui < len(units):
                            units[ui]()
                            ui += 1
                if sc == 0:
                    # phase C x loads: WAR on phase-A readers resolved by now
                    for k in range(NKT):
                        load_x(k, SC, S2 + SC, SC, "v")
                attn_oproj(sc, attn_sb)
            while ui < len(units):
                units[ui]()
                ui += 1
    ctx_lp.__exit__(None, None, None)
    nc.compile()
    return nc


_CACHE = {}
LAST_RESULTS = None


def _get_nc(causal: bool, has_bias: bool = False):
    key = (causal, has_bias)
    if key not in _CACHE:
        _CACHE[key] = _build(causal, has_bias)
    return _CACHE[key]


def _reference_host(hidden_state, attention_mask, wq, bq, wk, bk, wv, bv, wo, bo):
    """Exact numpy fallback for unexpected mask patterns."""
    x = hidden_state.astype(np.float64)
    q = (x @ wq.T.astype(np.float64) + bq).reshape(B, S, NH, HD).transpose(0, 2, 1, 3)
    k = (x @ wk.T.astype(np.float64) + bk).reshape(B, S, NH, HD).transpose(0, 2, 1, 3)
    v = (x @ wv.T.astype(np.float64) + bv).reshape(B, S, NH, HD).transpose(0, 2, 1, 3)
    sc = np.einsum("bhsd,bhtd->bhst", q, k) / np.sqrt(HD)
    sc = np.where(attention_mask, sc, -np.inf)
    sc -= sc.max(axis=-1, keepdims=True)
    e = np.exp(sc)
    p = e / e.sum(axis=-1, keepdims=True)
    o = np.einsum("bhst,bhtd->bhsd", p, v).transpose(0, 2, 1, 3).reshape(B, S, HID)
    return (o @ wo.T.astype(np.float64) + bo).astype(np.float32)


def kernel(hidden_state, attention_mask, wq, bq, wk, bk, wv, bv, wo, bo):
    global LAST_RESULTS
    hidden_state = np.asarray(hidden_state, dtype=np.float32)
    attention_mask = np.asarray(attention_mask, dtype=bool)
    wq, bq = np.asarray(wq, np.float32), np.asarray(bq, np.float32)
    wk, bk = np.asarray(wk, np.float32), np.asarray(bk, np.float32)
    wv, bv = np.asarray(wv, np.float32), np.asarray(bv, np.float32)
    wo, bo = np.asarray(wo, np.float32), np.asarray(bo, np.float32)

    tril = np.tril(np.ones((S, S), dtype=bool))
    if (attention_mask == tril).all():
        causal = True
    elif attention_mask.all():
        causal = False
    else:
        return _reference_host(
            hidden_state, attention_mask, wq, bq, wk, bk, wv, bv, wo, bo
        )

    mask_tri = np.triu(np.ones((TT, TT), dtype=np.float32))
    in_maps = []
    for c in range(N_CORES):
        b, g = c // 4, c % 4
        r0 = DPC * g
        in_maps.append(
            {
                "xT": np.ascontiguousarray(hidden_state[b].T),
                "wqT": np.ascontiguousarray(wq[r0 : r0 + DPC].T),
                "wkT": np.ascontiguousarray(wk[r0 : r0 + DPC].T),
                "wvT": np.ascontiguousarray(wv[r0 : r0 + DPC].T),
                "woT": np.ascontiguousarray(wo[:, r0 : r0 + DPC].T),
                "bq_r": np.ascontiguousarray(bq[r0 : r0 + DPC].reshape(1, DPC)),
                "bk_r": np.ascontiguousarray(bk[r0 : r0 + DPC].reshape(1, DPC)),
                "mask_tri": mask_tri,
            }
        )

    has_bias = bool(np.any(bq) or np.any(bk))
    nc = _get_nc(causal, has_bias)
    res = run_bass_kernel_spmd(nc, in_maps, list(range(N_CORES)))
    LAST_RESULTS = res

    out = np.zeros((B, S, HID), dtype=np.float32)
    for c in range(N_CORES):
        out[c // 4] += res.results[c]["outT"].T
    out += (bv @ wo.T + bo)[None, None, :]
    return out
